# revision 53
# baseline (speedup 1.0000x reference)
"""Multi-head causal attention (B=2, S=2048, D=1024, H=16) on 8 TRN2 NeuronCores.

Sharding: core c -> (head-group g = c//2 of 4 heads, batch half s = c%2).
Each core computes Q/K/V projections for its 4 heads over its batch element,
causal softmax attention, and a partial output projection (its 256 columns of
Wo). Host sums the 4 per-group bf16 partials for each batch element, adds bo.

Device layout notes:
- All matmuls run in bf16 (inputs/weights quantized host-side; Q/K/attn
  written back to bf16); accumulation stays fp32 in PSUM.
- Activations X are passed pre-transposed (X^T, [D, S]) so every projection
  contracts over the embed dim on the partition axis.
- Scores are computed transposed (S^T [k, q]) so the attention matmul
  (attn @ V) needs no transposes; softmax denominators come from an
  appended ones-column in V; normalization is reciprocal-from-PSUM (DVE)
  -> K=1 ones-matmul broadcast (PE) -> elementwise mult (DVE/Pool).
- The emission engine software-pipelines phases: projection / out-projection
  matmul chains are queued as filler closures and popped inside attention
  rows, so the PE never idles while the ACT exp stream (the per-row critical
  resource) catches up.
"""

import contextlib
import sys
from collections import deque

sys.path.insert(0, "/opt/trn_rl_repo")

import numpy as np

import concourse.bass as bass  # noqa: F401  (bass must import before bacc)
import concourse.mybir as mybir
from concourse import bacc
from concourse.bass_utils import run_bass_kernel_spmd
from concourse.tile import TileContext

F32 = mybir.dt.float32
F32R = mybir.dt.float32r
BF16 = mybir.dt.bfloat16
AF = mybir.ActivationFunctionType
ALU = mybir.AluOpType

B = 2
S = 2048            # sequence per batch element (= rows per core)
D = 1024            # embed dim
H = 16              # total heads
HD = 64             # head dim
DL = 256            # local dims per core (4 heads)
NI = D // 128       # 8 contraction tiles for projections
NQ = S // 512       # 4 query tiles of 512
NK = S // 128       # 16 key tiles of 128
SCALE = HD ** -0.5


def _build_nc(loop_iters=None, phases="full"):
    nc = bacc.Bacc()

    xq_d = nc.declare_dram_parameter("xq_t", [NQ, D, S // NQ], BF16,
                                     isOutput=False)
    xk_d = nc.declare_dram_parameter("xk_t", [NQ, D, S // NQ], BF16,
                                     isOutput=False)
    xv_d = nc.declare_dram_parameter("xv_t", [NQ, D, S // NQ], BF16,
                                     isOutput=False)
    wq_d = nc.declare_dram_parameter("wq_t", [D, DL], BF16, isOutput=False)
    wk_d = nc.declare_dram_parameter("wk_t", [D, DL], BF16, isOutput=False)
    wv_d = nc.declare_dram_parameter("wv_t", [D, DL], BF16, isOutput=False)
    wo_d = nc.declare_dram_parameter("wo_t", [DL, D], BF16, isOutput=False)
    bq_d = nc.declare_dram_parameter("bq", [DL, 1], F32, isOutput=False)
    bk_d = nc.declare_dram_parameter("bk", [DL, 1], F32, isOutput=False)
    bv_d = nc.declare_dram_parameter("bv_bc", [128, DL], F32, isOutput=False)
    mk_d = nc.declare_dram_parameter("masks", [128, 128], BF16, isOutput=False)
    on_d = nc.declare_dram_parameter("ones66", [66, 128], BF16, isOutput=False)
    oc_d = nc.declare_dram_parameter("ones_col", [128, NK, 1], BF16,
                                     isOutput=False)
    out_d = nc.declare_dram_parameter("out", [S, D], BF16, isOutput=True)

    with TileContext(nc) as tc:
        with tc.tile_pool(name="const", bufs=1) as cp, \
             tc.tile_pool(name="xpool", bufs=4) as xp, \
             tc.tile_pool(name="work", bufs=3) as wp, \
             tc.tile_pool(name="psum", bufs=8, space="PSUM") as pp:

            ET = mybir.EngineType
            loop_cm = (tc.For_i(0, loop_iters, 1,
                                hint_engines=(ET.PE, ET.DVE, ET.Activation,
                                              ET.SP, ET.Pool))
                       if loop_iters else contextlib.nullcontext())
            with loop_cm:
                # ---- persistent SBUF tensors ----
                wq_sb = cp.tile([128, NI * DL], BF16)
                wk_sb = cp.tile([128, NI * DL], BF16)
                wv_sb = cp.tile([128, NI * DL], BF16)
                wo_sb = cp.tile([128, 2 * D], BF16)
                qt_sb = cp.tile([128, 2 * S], BF16)   # Q^T: pair p cols [p*S:(p+1)*S]
                kt_sb = cp.tile([128, 2 * S], BF16)
                at_sb = cp.tile([128, 2 * S], BF16)   # attn out^T (normalized)
                va0 = cp.tile([128, NK * 65], BF16)   # head A of pair 0, +ones col 64
                va1 = cp.tile([128, NK * 65], BF16)
                vb0 = cp.tile([128, NK * 128], BF16)  # head B: col0=ones, 64:128=V
                vb1 = cp.tile([128, NK * 128], BF16)
                va = [va0, va1]
                vb = [vb0, vb1]
                mask_sb = cp.tile([128, 128], BF16)
                ones_sb = cp.tile([66, 128], BF16)
                bq_sb = cp.tile([128, 2], F32)
                bk_sb = cp.tile([128, 2], F32)
                bv_sb = cp.tile([128, DL], F32)

                # ---- phase 1: projections. DMAs are emitted immediately
                # (need-ordered); the PE chains are returned as filler
                # closures the attention rows pop between attnV steps. ----
                def project_steps(jn):
                    nsl = slice(jn * 512, (jn + 1) * 512)
                    xq_sl = xp.tile([128, NI * 512], BF16, tag="xq", bufs=2,
                                    name=f"xq_{jn}")
                    xk_sl = xp.tile([128, NI * 512], BF16, tag="xk", bufs=2,
                                    name=f"xk_{jn}")
                    xv_sl = xp.tile([128, NI * 512], BF16, tag="xv", bufs=2,
                                    name=f"xv_{jn}")

                    def dma_x(dst, src, hf, w):
                        # jn-major host tiling: each slab chunk is one fully
                        # sequential DRAM range
                        hi = slice(hf * w, (hf + 1) * w)
                        hr = slice(hf * w * 128, (hf + 1) * w * 128)
                        nc.sync.dma_start(
                            out=dst.rearrange("p (a n) -> p a n", n=512)[:, hi],
                            in_=src[jn, hr, :].rearrange("(a p) n -> p a n",
                                                         p=128))

                    def dma_w(w_sb, w_d, hf, w):
                        hi = slice(hf * w, (hf + 1) * w)
                        nc.sync.dma_start(
                            out=w_sb.rearrange("p (a m) -> p a m", m=DL)[:, hi],
                            in_=w_d.rearrange("(a p) m -> p a m", p=128)[:, hi])

                    if jn == 0:
                        # strict need-order: q chains stream xq quarters,
                        # then k chains stream xk, then the V wave streams xv
                        for hf in range(4):
                            dma_w(wq_sb, wq_d, hf, 2)
                            dma_x(xq_sl, xq_d, hf, 2)
                        for hf in range(4):
                            dma_w(wk_sb, wk_d, hf, 2)
                            dma_x(xk_sl, xk_d, hf, 2)
                            if hf == 0:
                                for p in range(2):
                                    nc.sync.dma_start(
                                        out=bq_sb[:, p:p + 1],
                                        in_=bq_d[p * 128:(p + 1) * 128, :])
                                    nc.sync.dma_start(
                                        out=bk_sb[:, p:p + 1],
                                        in_=bk_d[p * 128:(p + 1) * 128, :])
                        nc.sync.dma_start(
                            out=wv_sb.rearrange("p (a m) -> p a m", m=DL),
                            in_=wv_d.rearrange("(a p) m -> p a m", p=128))
                        nc.sync.dma_start(out=bv_sb, in_=bv_d[:])
                        for p in range(2):
                            nc.sync.dma_start(
                                out=va[p].rearrange(
                                    "q (m c) -> q m c", c=65)[:, :, 64:65],
                                in_=oc_d[:])
                            nc.sync.dma_start(
                                out=vb[p].rearrange(
                                    "q (m c) -> q m c", c=128)[:, :, 0:1],
                                in_=oc_d[:])
                        for hf in range(2):
                            dma_x(xv_sl, xv_d, hf, 4)
                    else:
                        for hf in range(2):
                            dma_x(xq_sl, xq_d, hf, 4)
                        for hf in range(2):
                            dma_x(xk_sl, xk_d, hf, 4)
                        dma_x(xv_sl, xv_d, 0, 8)
                    xq_t = [xq_sl[:, ji * 512:(ji + 1) * 512] for ji in range(NI)]
                    xk_t = [xk_sl[:, ji * 512:(ji + 1) * 512] for ji in range(NI)]
                    xv_t = [xv_sl[:, ji * 512:(ji + 1) * 512] for ji in range(NI)]

                    steps = []

                    # wave A: Q then K, per-pair [128,512] chains in the
                    # 2-bank "small" rotation; each chain is two filler steps
                    def qk_chain(xi, t):
                        x_t, w_sb, b_sb, o_sb = (
                            (xq_t, wq_sb, bq_sb, qt_sb),
                            (xk_t, wk_sb, bk_sb, kt_sb))[xi]
                        cell = {}

                        def h1():
                            ps = pp.tile([128, 512], F32, tag="small", bufs=2,
                                         name=f"ps{'qk'[xi]}_{jn}_{t}")
                            cell["ps"] = ps
                            for ji in range(4):
                                wsl = slice(ji * DL + t * 128,
                                            ji * DL + (t + 1) * 128)
                                nc.tensor.matmul(ps, w_sb[:, wsl], x_t[ji],
                                                 start=ji == 0, stop=False)

                        def h2():
                            ps = cell["ps"]
                            for ji in range(4, NI):
                                wsl = slice(ji * DL + t * 128,
                                            ji * DL + (t + 1) * 128)
                                nc.tensor.matmul(ps, w_sb[:, wsl], x_t[ji],
                                                 start=False, stop=ji == NI - 1)
                            dst = slice(t * S + jn * 512,
                                        t * S + (jn + 1) * 512)
                            nc.vector.tensor_scalar(o_sb[:, dst], ps,
                                                    b_sb[:, t:t + 1], None,
                                                    ALU.add)
                        return [h1, h2]

                    # q chains must complete before this tile's attention
                    # rows (marked required); k/v chains are only consumed
                    # late in the rows and may pop lazily inside them
                    for xi in range(2):
                        for t in range(2):
                            steps += [(h, xi == 0) for h in qk_chain(xi, t)]

                    # wave B: V projection, two [128,512] chains; each bank
                    # holds two 256-col sub-chains (start=True only on the
                    # first matmul touching the bank)
                    def v_chain(w):
                        cell = {}

                        def h1():
                            ps = pp.tile([128, 512], F32, tag="small", bufs=2,
                                         name=f"psv2_{jn}_{w}")
                            cell["ps"] = ps
                            for ji in range(4):
                                for c in range(2):
                                    u = w * 2 + c
                                    nc.tensor.matmul(
                                        ps[:, c * DL:(c + 1) * DL],
                                        xv_t[ji][:, u * 128:(u + 1) * 128],
                                        wv_sb[:, ji * DL:(ji + 1) * DL],
                                        start=(ji == 0 and c == 0), stop=False)

                        def h2():
                            ps = cell["ps"]
                            for ji in range(4, NI):
                                for c in range(2):
                                    u = w * 2 + c
                                    nc.tensor.matmul(
                                        ps[:, c * DL:(c + 1) * DL],
                                        xv_t[ji][:, u * 128:(u + 1) * 128],
                                        wv_sb[:, ji * DL:(ji + 1) * DL],
                                        start=False, stop=ji == NI - 1)
                            for c in range(2):
                                u = w * 2 + c
                                m = jn * 4 + u
                                for p in range(2):
                                    sa = slice(p * 128, p * 128 + 64)
                                    sb_ = slice(p * 128 + 64, p * 128 + 128)
                                    nc.vector.tensor_tensor(
                                        out=va[p][:, m * 65:m * 65 + 64],
                                        in0=ps[:, c * DL + sa.start:
                                               c * DL + sa.stop],
                                        in1=bv_sb[:, sa], op=ALU.add)
                                    nc.vector.tensor_tensor(
                                        out=vb[p][:, m * 128 + 64:
                                                  m * 128 + 128],
                                        in0=ps[:, c * DL + sb_.start:
                                               c * DL + sb_.stop],
                                        in1=bv_sb[:, sb_], op=ALU.add)
                        return [h1, h2]

                    for w in range(2):
                        steps += [(h, False) for h in v_chain(w)]
                    return steps

                # ---- phase 2: causal attention (head pairs packed on
                # partitions); pops filler between attnV steps ----
                def attention(p, jq, fill, mid=None):
                    qsl = slice(p * S + jq * 512, p * S + (jq + 1) * 512)
                    nk = 4 * jq + 4
                    ps_oa = pp.tile([65, 512], F32, tag="acc", bufs=2,
                                    name=f"oa{p}_{jq}")
                    ps_ob = pp.tile([128, 512], F32, tag="acc", bufs=2,
                                    name=f"ob{p}_{jq}")

                    def scores(jk):
                        d = jk - 4 * jq
                        c0 = 128 * d if d > 0 else 0  # first causally-valid col
                        ksl = slice(p * S + jk * 128, p * S + (jk + 1) * 128)
                        qsl_v = slice(p * S + jq * 512 + c0,
                                      p * S + (jq + 1) * 512)
                        ps_s2 = pp.tile([128, 1024], F32, tag="s2", bufs=2,
                                        name=f"s2{p}_{jq}_{jk}")
                        nc.tensor.matmul(ps_s2[:, c0:512], kt_sb[0:64, ksl],
                                         qt_sb[0:64, qsl_v],
                                         start=True, stop=True,
                                         tile_position=(0, 0))
                        nc.tensor.matmul(ps_s2[:, 512 + c0:1024],
                                         kt_sb[64:128, ksl],
                                         qt_sb[64:128, qsl_v],
                                         start=True, stop=True,
                                         tile_position=(64, 0))
                        e2 = wp.tile([128, 1024], BF16, tag="e2", bufs=8,
                                     name=f"e2{p}_{jq}_{jk}")
                        s2v = ps_s2.rearrange("q (h n) -> q h n", n=512)[:, :, c0:]
                        e2v = e2.rearrange("q (h n) -> q h n", n=512)[:, :, c0:]
                        nc.scalar.activation(e2v, s2v, AF.Exp, scale=SCALE)
                        if d >= 0:  # diagonal block: the masked triangle
                            # spans exactly cols [c0, c0+128); beyond that
                            # q - k >= 128(d+1) - 127 > 128d, i.e. all valid
                            mkm = mask_sb[:, :]
                            for h in range(2):
                                e2m = e2[:, h * 512 + c0:h * 512 + c0 + 128]
                                nc.gpsimd.tensor_tensor(out=e2m, in0=e2m,
                                                        in1=mkm, op=ALU.mult)
                        return e2, c0

                    # software-pipelined: scores(jk+1) is emitted before
                    # attnV(jk) so attnV never head-of-line-waits on the exp
                    # of its own tile; one filler step lands after each attnV
                    nxt = scores(0)
                    for jk in range(nk):
                        cur, nxt = nxt, scores(jk + 1) if jk + 1 < nk else None
                        e2, c0 = cur
                        if jk == 0:
                            # the previous norm MUST be emitted before any
                            # filler pop: out_proj steps in the queue read
                            # the at_sb columns that norm writes
                            if mid is not None:
                                mid()
                            if fill:
                                # cover the first exp's latency: the row has
                                # no primed pipeline yet
                                fill.popleft()[0]()
                        st, sp = jk == 0, jk == nk - 1
                        nc.tensor.matmul(ps_oa[:, c0:512],
                                         va[p][:, jk * 65:(jk + 1) * 65],
                                         e2[:, c0:512], start=st, stop=sp)
                        nc.tensor.matmul(ps_ob[:, c0:512],
                                         vb[p][:, jk * 128:(jk + 1) * 128],
                                         e2[:, 512 + c0:1024], start=st, stop=sp)
                        if jk > 0 and fill:
                            fill.popleft()[0]()

                    # softmax denominators: reciprocal straight out of PSUM
                    # (DVE); the deferred PE ones-matmul broadcast (`norm`) is
                    # emitted later, behind other PE work, so the recips never
                    # head-of-line-block the PE queue
                    rsa = wp.tile([65, 512], BF16, tag="rsa", name=f"rsa{p}_{jq}")
                    rsb = wp.tile([1, 512], BF16, tag="rsb", name=f"rsb{p}_{jq}")
                    with nc.allow_low_precision(
                            reason="1/denominator in bf16 is within budget"):
                        nc.vector.reciprocal(rsa[64:65, :], ps_oa[64:65, :])
                        nc.vector.reciprocal(rsb, ps_ob[0:1, :])
                    # unnormalized numerators land in at_sb; DVE serializes
                    # these behind the recips but ACT must stay exp-only
                    nc.vector.tensor_copy(at_sb[0:64, qsl], ps_oa[0:64, :])
                    nc.vector.tensor_copy(at_sb[64:128, qsl], ps_ob[64:128, :])

                    def norm():
                        ps_ba = pp.tile([128, 512], F32, tag="small", bufs=2,
                                        name=f"ba{p}_{jq}")
                        ps_bb = pp.tile([128, 512], F32, tag="small", bufs=2,
                                        name=f"bb{p}_{jq}")
                        nc.tensor.matmul(ps_ba, ones_sb[64:65, :],
                                         rsa[64:65, :], start=True, stop=True)
                        nc.tensor.matmul(ps_bb, ones_sb[0:1, :], rsb,
                                         start=True, stop=True)
                        nc.vector.tensor_tensor(out=at_sb[0:64, qsl],
                                                in0=ps_ba[0:64, :],
                                                in1=at_sb[0:64, qsl],
                                                op=ALU.mult)
                        nc.vector.tensor_tensor(out=at_sb[64:128, qsl],
                                                in0=ps_bb[64:128, :],
                                                in1=at_sb[64:128, qsl],
                                                op=ALU.mult)
                    return norm

                # ---- phase 3: partial output projection; filler closures ----
                def out_proj_step(jn2):
                    def step():
                        o_sb = wp.tile([128, 1024], BF16, tag="osb",
                                       name=f"osb{jn2}")
                        for jo in range(2):
                            ps_o = pp.tile([128, 512], F32, tag="small",
                                           bufs=2, name=f"po{jn2}_{jo}")
                            for p in range(2):
                                nc.tensor.matmul(
                                    ps_o,
                                    at_sb[:, p * S + jn2 * 128:
                                          p * S + (jn2 + 1) * 128],
                                    wo_sb[:, p * D + jo * 512:
                                          p * D + (jo + 1) * 512],
                                    start=(p == 0), stop=(p == 1))
                            nc.vector.tensor_copy(
                                o_sb[:, jo * 512:(jo + 1) * 512], ps_o)
                        # Pool's idle DMA ring: output never queues behind
                        # the input slabs on the sync ring
                        nc.gpsimd.dma_start(
                            out=out_d[jn2 * 128:(jn2 + 1) * 128, :],
                            in_=o_sb)
                    return step

                fill = deque()
                norm_prev = None
                for jq in range(NQ):
                    if jq == 0:
                        for st, _ in project_steps(0):
                            st()
                        # phase-2/3 constants load once phase 1 is underway
                        nc.sync.dma_start(out=mask_sb, in_=mk_d[:])
                        nc.sync.dma_start(out=ones_sb, in_=on_d[:])
                    if jq + 1 < NQ:
                        fill.extend(project_steps(jq + 1))  # DMAs start now
                    if jq == 0:
                        # wo is first needed by out_proj(0) (~mid of the jq=1
                        # rows) -- stream it after the jq=1 slabs
                        nc.sync.dma_start(
                            out=wo_sb.rearrange("p (a m) -> p a m", m=D),
                            in_=wo_d.rearrange("(a p) m -> p a m", p=128))
                    if phases == "p1":
                        while fill:
                            fill.popleft()[0]()
                        continue
                    n0 = attention(0, jq, fill, mid=norm_prev)
                    n1 = attention(1, jq, fill, mid=n0)
                    # drain only what the next rows hard-require (q chains);
                    # k/v chains keep popping lazily inside the next rows
                    while any(r for _, r in fill):
                        fill.popleft()[0]()
                    if jq + 1 < NQ:
                        norm_prev = n1
                        if phases == "full":
                            fill.extend((out_proj_step(jn2), False)
                                        for jn2 in range(4 * jq, 4 * jq + 4))
                        elif jq == 0:
                            fill.append((out_proj_step(0), False))
                    else:
                        n1()
                        if phases == "full":
                            for jn2 in range(4 * jq, 4 * jq + 4):
                                out_proj_step(jn2)()
                while fill:
                    fill.popleft()[0]()

                if phases == "p1":  # dummy out write so `out` has a producer
                    dmy = wp.tile([128, 512], BF16, tag="osb", name="dmy")
                    nc.vector.tensor_copy(dmy, qt_sb[:, 0:512])
                    nc.sync.dma_start(out=out_d[0:128, 0:512], in_=dmy)
    nc.finalize()
    return nc


_NC = {}


def _get_nc(loop_iters=None, phases="full"):
    key = (loop_iters, phases)
    if key not in _NC:
        _NC[key] = _build_nc(loop_iters, phases)
    return _NC[key]


def _host_masks():
    # every causal diagonal block reduces to the same [k, c] triangle:
    # within block d the first valid column is c0 = 128d, and
    # q - k = (c0 + c_local) - (128d + k_local) = c_local - k_local
    kl = np.arange(128)[:, None]
    ql = np.arange(128)[None, :]
    return (ql >= kl).astype(np.float32)


def build_in_maps(query, key_in, value, Wq, bq, Wk, bk, Wv, bv, Wo, bo):
    query = np.asarray(query, dtype=np.float32)
    key_in = np.asarray(key_in, dtype=np.float32)
    value = np.asarray(value, dtype=np.float32)
    Wq = np.asarray(Wq, dtype=np.float32)
    Wk = np.asarray(Wk, dtype=np.float32)
    Wv = np.asarray(Wv, dtype=np.float32)
    Wo = np.asarray(Wo, dtype=np.float32)
    bq = np.asarray(bq, dtype=np.float32)
    bk = np.asarray(bk, dtype=np.float32)
    bv = np.asarray(bv, dtype=np.float32)
    bo = np.asarray(bo, dtype=np.float32)

    import ml_dtypes
    bf16 = ml_dtypes.bfloat16
    masks = np.ascontiguousarray(_host_masks()).astype(bf16)
    ones66 = np.ones((66, 128), dtype=np.float32).astype(bf16)
    ones_col = np.ones((128, NK, 1), dtype=np.float32).astype(bf16)
    def _tile_x(x):  # [S, D] -> jn-major [NQ, D, 512] transposed tiles
        return np.ascontiguousarray(
            x.T.reshape(D, NQ, S // NQ).transpose(1, 0, 2)).astype(bf16)

    xq = [_tile_x(query[s]) for s in range(B)]
    xk = [_tile_x(key_in[s]) for s in range(B)]
    xv = [_tile_x(value[s]) for s in range(B)]

    in_maps = []
    for c in range(8):
        g, s = c // 2, c % 2
        dsl = slice(g * DL, (g + 1) * DL)
        in_maps.append({
            "xq_t": xq[s],
            "xk_t": xk[s],
            "xv_t": xv[s],
            "wq_t": np.ascontiguousarray(Wq[dsl, :].T).astype(bf16),
            "wk_t": np.ascontiguousarray(Wk[dsl, :].T).astype(bf16),
            "wv_t": np.ascontiguousarray(Wv[dsl, :].T).astype(bf16),
            "wo_t": np.ascontiguousarray(Wo[:, dsl].T).astype(bf16),
            "bq": np.ascontiguousarray(bq[dsl, None]),
            "bk": np.ascontiguousarray(bk[dsl, None]),
            "bv_bc": np.ascontiguousarray(
                np.broadcast_to(bv[None, dsl], (128, DL))),
            "masks": masks,
            "ones66": ones66,
            "ones_col": ones_col,
        })
    return in_maps


def kernel(query, key_in, value, Wq, bq, Wk, bk, Wv, bv, Wo, bo):
    bo = np.asarray(bo, dtype=np.float32)
    in_maps = build_in_maps(query, key_in, value, Wq, bq, Wk, bk, Wv, bv, Wo, bo)
    nc = _get_nc()
    res = run_bass_kernel_spmd(nc, in_maps, core_ids=list(range(8)))

    out = np.zeros((B, S, D), dtype=np.float32)
    for c in range(8):
        s = c % 2
        out[s] += res.results[c]["out"].astype(np.float32)
    out += bo[None, None, :]
    return out
